# revision 13
# baseline (speedup 1.0000x reference)
"""Trainium2 Bass kernel for CustomMHA prefill (B=2, S=2048, D=1024, H=16).

Sharding: batch*heads across 8 cores — core m computes heads {2m, 2m+1} for
both batch rows.  qkv is row-sharded per core (the rows of this core's heads),
wo is column-sharded (the o-channels of this core's heads); each core emits a
partial output projection and the host sums the 8 partials (the "all-reduce"
of the tensor-parallel output projection happens on host at gather time).

Per-core device pipeline (all layouts transposed so no big on-chip transpose
is ever needed; PE matmul computes out = lhsT.T @ rhs):
  qT/kT/vT [2h*64, S]  = W.T-chunks.T @ xT          (xT fed pre-transposed)
  scoresT  [s_k, s_q]  = kT_tile.T @ qT_slice        (K=dh=64)
  expT     = ACT exp(scale=1/8), causal mask via gpsimd affine_select
  oT      += v_nat_aug.T @ expT                      (ones column appended to v
                                                      gives the softmax denom as
                                                      an extra psum row for free)
  normalize: denom -> reciprocal (straight off the psum row) -> K=1 PE outer
             product broadcasts it across partitions -> one DVE mul
  pout     = per-head K=64 matmul pair accumulating in psum (partial o-channels)
Scores are tiny (|s|<~1), so no max-subtraction is needed in softmax.
Scores/exp are processed in [128,1024] two-bank pairs to halve ACT op count;
kv-cache outputs leave in transposed [head, dh, S] layout (8KB DMA lines, no
PE transpose) and the host transposes them back at gather time.  Input loads
ride the ACT HWDGE queue, stores the SP queue, so neither blocks the other.
"""

import numpy as np

import concourse.bacc as bacc
import concourse.tile as tile
import concourse.bass_utils as bass_utils
from concourse import mybir
from concourse.masks import make_identity

B, D, H, DH = 2, 1024, 16, 64
NCORES = 8
F32 = mybir.dt.float32
F32R = mybir.dt.float32r
EXP = mybir.ActivationFunctionType.Exp
COPY = mybir.ActivationFunctionType.Copy

_CACHE = {}


def build(S):
    NS = S // 512   # 512-wide q slices
    NT = S // 128   # 128-wide s tiles

    nc = bacc.Bacc("TRN2", target_bir_lowering=False, debug=False)

    xt = nc.dram_tensor("xt", [B, 8, 128, S], F32, kind="ExternalInput").ap()
    wqkv = nc.dram_tensor("wqkv", [3, 8, 128, 128], F32, kind="ExternalInput").ap()
    wo_sl = nc.dram_tensor("wo_sl", [128, D], F32, kind="ExternalInput").ap()
    ones_c = nc.dram_tensor("ones_c", [128, 64], F32, kind="ExternalInput").ap()
    k_out = nc.dram_tensor("k_out", [2 * B, DH, S], F32, kind="ExternalOutput").ap()
    v_out = nc.dram_tensor("v_out", [2 * B, DH, S], F32, kind="ExternalOutput").ap()
    pout = nc.dram_tensor("pout", [B, S, D], F32, kind="ExternalOutput").ap()

    with tile.TileContext(nc) as tc:
        with (
            tc.tile_pool(name="const", bufs=1) as constp,
            tc.tile_pool(name="xp", bufs=1) as xp,
            tc.tile_pool(name="qkv", bufs=2) as qkvp,
            tc.tile_pool(name="vsb", bufs=2 * NT) as vsbp,
            tc.tile_pool(name="expp", bufs=6) as expp,
            tc.tile_pool(name="work", bufs=3) as workp,
            tc.tile_pool(name="ps_mm", bufs=2, space="PSUM") as ps_mm,
            tc.tile_pool(name="ps_sc", bufs=2, space="PSUM") as ps_sc,
            tc.tile_pool(name="ps_ot", bufs=2, space="PSUM") as ps_ot,
        ):
            ident = constp.tile([128, 128], F32)
            make_identity(nc, ident)
            pat = constp.tile([65, 64], F32R)
            nc.scalar.dma_start(out=pat[64:65, :], in_=ones_c[0:1, :].bitcast(F32R))
            ones_sb = constp.tile([128, 1], F32R)
            nc.scalar.dma_start(out=ones_sb, in_=ones_c[:, 0:1].bitcast(F32R))
            w_sb = constp.tile([128, 3, 8, 128], F32R)
            nc.scalar.dma_start(out=w_sb, in_=wqkv.rearrange("d c p m -> p d c m").bitcast(F32R))
            wo_sb = constp.tile([128, D], F32R)
            nc.scalar.dma_start(out=wo_sb, in_=wo_sl.bitcast(F32R))
            # second copy of wo rows 64:128 at partition base 0 (matmul needs
            # lhsT and rhs at the same base partition; engines can't shift)
            wo_sb2 = constp.tile([64, D], F32R)
            nc.scalar.dma_start(out=wo_sb2, in_=wo_sl[64:128, :].bitcast(F32R))

            for b in range(B):
                x_sb = xp.tile([128, 8, S], F32R, tag="x")
                for c in range(8):
                    nc.scalar.dma_start(out=x_sb[:, c, :], in_=xt[b, c].bitcast(F32R))

                # --- projections: qT/kT (f32r, used as matmul inputs), vT (f32, transposed) ---
                qT = qkvp.tile([128, S], F32R, tag="qT")
                kT = qkvp.tile([128, S], F32R, tag="kT")
                vT = qkvp.tile([128, S], F32, tag="vT")
                for di, dst in enumerate([qT, kT, vT]):
                    for ns in range(NS):
                        mm_ps = ps_mm.tile([128, 512], F32, tag="mm")
                        for c in range(8):
                            nc.tensor.matmul(
                                mm_ps,
                                lhsT=w_sb[:, di, c, :],
                                rhs=x_sb[:, c, ns * 512:(ns + 1) * 512],
                                start=(c == 0),
                                stop=(c == 7),
                            )
                        if di < 2:
                            nc.scalar.activation(out=dst[:, ns * 512:(ns + 1) * 512], in_=mm_ps, func=COPY)
                        else:
                            nc.vector.tensor_copy(out=dst[:, ns * 512:(ns + 1) * 512], in_=mm_ps)

                # --- kv-cache outputs straight from the T-layout (8KB/partition lines) ---
                for hl in range(2):
                    nc.sync.dma_start(out=k_out[2 * b + hl], in_=kT[hl * 64:(hl + 1) * 64, :].bitcast(F32))
                    nc.sync.dma_start(out=v_out[2 * b + hl], in_=vT[hl * 64:(hl + 1) * 64, :])
                # --- v natural layout (PE transpose) + ones columns for the denominator trick ---
                v_tiles = []
                for t in range(NT):
                    tr_full = ps_mm.tile([128, 512], F32, tag="mm", name="tr_full")
                    tr_ps = tr_full[:, 0:128]
                    nc.tensor.transpose(tr_ps, vT[:, t * 128:(t + 1) * 128], ident)
                    v_sb = vsbp.tile([128, 130], F32R, tag="v")
                    nc.scalar.activation(out=v_sb[:, 0:64], in_=tr_ps[:, 0:64], func=COPY)
                    nc.scalar.activation(out=v_sb[:, 65:129], in_=tr_ps[:, 64:128], func=COPY)
                    nc.vector.tensor_copy(out=v_sb[:, 64:65], in_=ones_sb)
                    nc.vector.tensor_copy(out=v_sb[:, 129:130], in_=ones_sb)
                    v_tiles.append(v_sb)

                # --- attention ---
                for j in range(NS):
                    ktiles = 4 * j + 4
                    oT_list = []
                    for h in range(2):
                        oT_ps = ps_ot.tile([128, 512], F32, tag="ot")
                        for t2 in range(0, ktiles, 2):
                            s2_ps = ps_sc.tile([128, 1024], F32, tag="sc")
                            for dt in range(2):
                                t = t2 + dt
                                nc.tensor.matmul(
                                    s2_ps[:, dt * 512:(dt + 1) * 512],
                                    lhsT=kT[h * 64:(h + 1) * 64, t * 128:(t + 1) * 128],
                                    rhs=qT[h * 64:(h + 1) * 64, j * 512:(j + 1) * 512],
                                    start=True,
                                    stop=True,
                                )
                            e2_sb = expp.tile([128, 1024], F32R, tag="e")
                            nc.scalar.activation(out=e2_sb, in_=s2_ps, func=EXP, scale=0.125)
                            for dt in range(2):
                                t = t2 + dt
                                e_sb = e2_sb[:, dt * 512:(dt + 1) * 512]
                                if t >= 4 * j:
                                    # keep iff s_k <= s_q:  -p + f + (q0 - k0) >= 0
                                    nc.gpsimd.affine_select(
                                        out=e_sb,
                                        in_=e_sb,
                                        compare_op=mybir.AluOpType.is_ge,
                                        fill=0.0,
                                        base=512 * j - 128 * t,
                                        channel_multiplier=-1,
                                        pattern=[[1, 512]],
                                    )
                                # lhsT=[v_h|ones] -> channels at psum rows 0:64, denom at row 64
                                lhsT_v = v_tiles[t][:, 0:65] if h == 0 else v_tiles[t][:, 65:130]
                                nc.tensor.matmul(
                                    oT_ps[0:65, :], lhsT=lhsT_v, rhs=e_sb,
                                    start=(t == 0), stop=(t == ktiles - 1),
                                )
                        oT_list.append(oT_ps)

                    oT_norms = []
                    for h in range(2):
                        oT_raw = workp.tile([128, 512], F32, tag="oraw")
                        nc.vector.tensor_copy(out=oT_raw[0:64, :], in_=oT_list[h][0:64, :])
                        dr_sb = workp.tile([128, 512], F32R, tag="dr")
                        with nc.allow_low_precision(reason="f32r has ~19-bit mantissa; plenty for softmax denom"):
                            nc.vector.reciprocal(out=dr_sb[64:65, :], in_=oT_list[h][64:65, :])
                        r2_ps = ps_mm.tile([128, 512], F32, tag="mm")
                        nc.tensor.matmul(r2_ps[0:64, :], lhsT=pat[64:65, :], rhs=dr_sb[64:65, :], start=True, stop=True)
                        oT_norm = workp.tile([128, 512], F32R, tag="on")
                        nc.vector.tensor_mul(out=oT_norm[0:64, :], in0=oT_raw[0:64, :], in1=r2_ps[0:64, :])
                        oT_norms.append(oT_norm)

                    for tt in range(4):
                        row0 = (4 * j + tt) * 128
                        for nh in range(2):
                            p_ps = ps_mm.tile([128, 512], F32, tag="mm")
                            nc.tensor.matmul(
                                p_ps,
                                lhsT=oT_norms[0][0:64, tt * 128:(tt + 1) * 128],
                                rhs=wo_sb[0:64, nh * 512:(nh + 1) * 512],
                                start=True,
                                stop=False,
                            )
                            nc.tensor.matmul(
                                p_ps,
                                lhsT=oT_norms[1][0:64, tt * 128:(tt + 1) * 128],
                                rhs=wo_sb2[:, nh * 512:(nh + 1) * 512],
                                start=False,
                                stop=True,
                            )
                            p_sb = workp.tile([128, 512], F32, tag="po")
                            nc.vector.tensor_copy(out=p_sb, in_=p_ps)
                            nc.sync.dma_start(out=pout[b, row0:row0 + 128, nh * 512:(nh + 1) * 512], in_=p_sb)

    nc.compile()
    return nc


def shard_inputs(x, qkv, wo, S):
    """Build the 8 per-core input maps (host-side shard/layout prep)."""
    x = np.ascontiguousarray(np.asarray(x, dtype=np.float32))
    qkv = np.ascontiguousarray(np.asarray(qkv, dtype=np.float32))
    wo = np.ascontiguousarray(np.asarray(wo, dtype=np.float32))
    xt = np.ascontiguousarray(x.transpose(0, 2, 1)).reshape(B, 8, 128, S)
    ones_arr = np.ones((128, 64), dtype=np.float32)
    in_maps = []
    for m in range(NCORES):
        r0 = 128 * m
        blocks = []
        for d in range(3):
            w = qkv[d * D + r0: d * D + r0 + 128]          # [128, D]
            blocks.append(np.ascontiguousarray(w.T).reshape(8, 128, 128))
        wqkv_m = np.ascontiguousarray(np.stack(blocks))     # [3, 8, 128, 128]
        wo_m = np.ascontiguousarray(wo[:, r0:r0 + 128].T)   # [128, D]
        in_maps.append({"xt": xt, "wqkv": wqkv_m, "wo_sl": wo_m, "ones_c": ones_arr})
    return in_maps


def run(x, qkv, wo, S=2048, trace=False):
    if S not in _CACHE:
        _CACHE[S] = build(S)
    nc = _CACHE[S]
    in_maps = shard_inputs(x, qkv, wo, S)
    res = bass_utils.run_bass_kernel_spmd(nc, in_maps, core_ids=list(range(NCORES)), trace=trace)

    out = np.zeros((B, S, D), dtype=np.float32)
    k_heads = np.empty((B * H, S, DH), dtype=np.float32)
    v_heads = np.empty((B * H, S, DH), dtype=np.float32)
    for m in range(NCORES):
        r = res.results[m]
        out += r["pout"]
        for b in range(B):
            for hl in range(2):
                k_heads[b * H + 2 * m + hl] = r["k_out"][2 * b + hl].T
                v_heads[b * H + 2 * m + hl] = r["v_out"][2 * b + hl].T
    return (out, k_heads, v_heads), res


def kernel(x, qkv, wo):
    (out, k_heads, v_heads), _ = run(x, qkv, wo, S=2048, trace=False)
    return out, k_heads, v_heads
